# revision 1
# baseline (speedup 1.0000x reference)
"""GCN (nn_GCNModel) Trainium2 kernel — 8 NeuronCores, SPMD.

Design:
  - Shard nodes/edges by graph (graph_id sorted -> contiguous node ranges per core).
  - Feature-major on-chip state: xT [H(2 part-chunks 128+72), NODES_PAD] bf16.
  - Per layer: y = x @ conv_W computed locally (node-major tiles), written to DRAM,
    AllGather -> full y table [8*NODES_PAD, 256] bf16 (rows padded to 512B).
  - Edge aggregation: dma_gather (int16 idx, 32768-row chunks, 4 SWDGE queues)
    pulls y[src] rows for local edges (sorted by (tile-group, chunk, dst-tile)),
    then a 0/1 S-matrix matmul on the tensor engine segment-sums them per
    128-node dst tile: agg = S^T @ gathered  (PSUM fp32).
  - h = relu(agg + conv_b) (transposed to feature-major) + relu(x @ res_W + res_b).
  - BatchNorm over all nodes: per-partition (feature) sums via ACT accum_out /
    tensor_tensor_reduce, AllReduce of [sum, sumsq], ghost-column correction for
    padded fake nodes, then x = h*scale + shift in place.
  - Pooling: per-graph segment-sum via 0/1 P-matrix matmuls; readout MLP on PE.
"""
import math
import os
import numpy as np
import ml_dtypes

import concourse.bass as bass
import concourse.bacc as bacc
import concourse.mybir as mybir
import concourse.tile as tile
from concourse.library_config import mlp as _mlp_lib
from concourse.masks import make_identity
from concourse.bass_utils import run_bass_kernel_spmd

BF16 = mybir.dt.bfloat16
F32 = mybir.dt.float32
I16 = mybir.dt.int16

NCORES = 8
P = 128
H = 200
DIN = 74
L = 5
HP = 256          # padded feature row (bf16 -> 512B, %256B for dma_gather)
CHUNK = 32768     # int16 index range per gather chunk
TG = 2            # node-tiles per gather group
EPS = 1e-5
FCH = [(0, 128), (128, 72)]   # feature chunks (offset, width)


# --- patch: partition DMASW sem lanes by SWDGE queue so multi-queue dma_gather
# keeps each DMA-completion semaphore locked to a single queue (Tile's default
# round-robin assigns lanes in scheduled order, which mixes queues on a lane).
import concourse.tile_sem_assignment as _tsa
import concourse.bass_isa as _bass_isa

if not getattr(_tsa.TileClockTick, "_gcn_queue_patch", False):
    _orig_assign_tick = _tsa.TileClockTick._assign_tick

    def _patched_assign_tick(self, inst):
        if (isinstance(inst, _tsa.DMAInst)
                and not isinstance(inst, _bass_isa.UserSyncedRemoteDMADescs)
                and inst.engine == mybir.EngineType.Pool):
            q = int(getattr(inst, "queue_num", 0) or 0) % 4
            if not hasattr(self, "_gcn_qtgl"):
                self._gcn_qtgl = {}
            tgl = self._gcn_qtgl.get(q, 0)
            self._gcn_qtgl[q] = tgl ^ 1
            self.next_sw_dma_idx = (q + 4 * tgl) % self.swdge_sem_count
        return _orig_assign_tick(self, inst)

    _tsa.TileClockTick._assign_tick = _patched_assign_tick
    _tsa.TileClockTick._gcn_queue_patch = True

AF = mybir.ActivationFunctionType
ALU = mybir.AluOpType


def _ceil(a, b):
    return -(-a // b)


def _preprocess(node_feat, src, dst, graph_id, **kw):
    N = node_feat.shape[0]
    G = int(kw["G"])
    src = np.asarray(src).astype(np.int64)
    dst = np.asarray(dst).astype(np.int64)
    graph_id = np.asarray(graph_id).astype(np.int64)

    counts = np.bincount(graph_id, minlength=G)
    gstart = np.zeros(G + 1, np.int64)
    gstart[1:] = np.cumsum(counts)

    # graph-boundary cuts closest to uniform node split
    g_cuts = [0]
    for c in range(1, NCORES):
        target = round(c * N / NCORES)
        g = int(np.searchsorted(gstart, target))
        g = min(g, G)
        if g > 0 and abs(int(gstart[g - 1]) - target) <= abs(int(gstart[g]) - target):
            g -= 1
        g = max(g, g_cuts[-1])
        g_cuts.append(g)
    g_cuts.append(G)
    g_lo = np.array(g_cuts[:-1])
    g_hi = np.array(g_cuts[1:])
    n_lo = gstart[g_lo]
    n_hi = gstart[g_hi]
    n_count = n_hi - n_lo

    NODES_PAD = _ceil(int(n_count.max()), 512) * 512
    NT = NODES_PAD // P
    TABLE_ROWS = NCORES * NODES_PAD
    NCHUNK = _ceil(TABLE_ROWS, CHUNK)
    NF_TOTAL = TABLE_ROWS - N  # global fake-node count

    G_CORE = g_hi - g_lo
    G_PAD = _ceil(int(G_CORE.max()), P) * P
    GT = G_PAD // P

    # --- edges ---
    bounds = n_lo  # n_lo[0] == 0
    src_core = np.searchsorted(bounds, src, side="right") - 1
    dst_core = np.searchsorted(bounds, dst, side="right") - 1
    table_row = src_core * NODES_PAD + (src - n_lo[src_core])
    e_chunk = table_row // CHUNK
    e_local = table_row % CHUNK

    # per-core per-(chunk,tile) counts -> uniform K grid
    ecnt = np.zeros((NCORES, NCHUNK, NT), np.int64)
    dst_local_all = dst - n_lo[dst_core]
    t_all = dst_local_all // P
    np.add.at(ecnt, (dst_core, e_chunk, t_all), 1)
    K = _ceil(ecnt.max(axis=0), 128)
    # ensure every tile has >=1 block so its psum gets written
    empty_t = K.sum(axis=0) == 0
    K[0][empty_t] = 1

    # column enumeration: (tile-group, chunk, tile, k)
    colbase = np.full((NCHUNK, NT), -1, np.int64)
    groups = []
    col = 0
    for tg0 in range(0, NT, TG):
        tg_tiles = list(range(tg0, min(tg0 + TG, NT)))
        tg_col0 = col
        c_slices = []
        tile_blocks = {t: [] for t in tg_tiles}
        for c in range(NCHUNK):
            c0 = col
            for t in tg_tiles:
                colbase[c, t] = col if K[c, t] > 0 else -1
                for k in range(int(K[c, t])):
                    tile_blocks[t].append(col)
                    col += 1
            c_slices.append((c, c0, col - c0))
        groups.append(dict(col0=tg_col0, ncols=col - tg_col0,
                           c_slices=c_slices, tile_blocks=tile_blocks))
    NCOL = col

    # --- per-core S + gather idx ---
    S_all, idx_all, nf_all, P_all = [], [], [], []

    # pooling block grid (union over cores)
    t_lo_g = np.full(GT, 10 ** 9, np.int64)
    t_hi_g = np.full(GT, -1, np.int64)
    gsl_local = []
    for core in range(NCORES):
        gsl = gstart[g_lo[core]:g_hi[core] + 1] - n_lo[core]  # local graph starts
        gsl_local.append(gsl)
        for gt in range(GT):
            glo = gt * P
            ghi = min((gt + 1) * P, int(G_CORE[core]))
            if glo >= ghi:
                continue
            lo = int(gsl[glo])
            hi = int(gsl[ghi])
            if hi <= lo:
                continue
            t_lo_g[gt] = min(t_lo_g[gt], lo // P)
            t_hi_g[gt] = max(t_hi_g[gt], _ceil(hi, P))
    pool_blocks = []  # list of (gt, t)
    for gt in range(GT):
        if t_hi_g[gt] < 0:
            pool_blocks.append((gt, 0))
        else:
            for t in range(int(t_lo_g[gt]), int(t_hi_g[gt])):
                pool_blocks.append((gt, t))
    NPB = len(pool_blocks)
    pool_by_gt = {}
    for b, (gt, t) in enumerate(pool_blocks):
        pool_by_gt.setdefault(gt, []).append((b, t))

    for core in range(NCORES):
        sel = dst_core == core
        e_c = e_chunk[sel]
        e_l = e_local[sel]
        d_loc = dst[sel] - n_lo[core]
        t_e = d_loc // P
        tg_e = t_e // TG
        order = np.lexsort((d_loc, t_e, e_c, tg_e))
        e_c, e_l, d_loc, t_e = e_c[order], e_l[order], d_loc[order], t_e[order]

        # rank within (chunk, tile) segment
        seg_key = e_c * NT + t_e
        # edges are sorted so equal seg_keys are consecutive within a tg, and
        # (c,t) pairs don't repeat across tgs (t determines tg)
        change = np.empty(len(seg_key), bool)
        if len(seg_key):
            change[0] = True
            change[1:] = seg_key[1:] != seg_key[:-1]
        seg_start = np.maximum.accumulate(np.where(change, np.arange(len(seg_key)), 0))
        rank = np.arange(len(seg_key)) - seg_start

        col_e = colbase[e_c, t_e] + rank // 128
        r_e = rank % 128

        idx_flat = np.zeros(NCOL * 128, np.int16)
        idx_flat[col_e * 128 + r_e] = e_l.astype(np.int16)
        # wrap: position q -> [q%16, q//16], replicated across 8 groups of 16
        wrapped = idx_flat.reshape(NCOL * 8, 16).T  # [16, NCOL*8]
        idx_arr = np.tile(wrapped, (8, 1))  # [128, NCOL*8]
        idx_all.append(np.ascontiguousarray(idx_arr))

        S = np.zeros((NCOL, 128, 128), ml_dtypes.bfloat16)
        S[col_e, r_e, d_loc - t_e * P] = 1.0
        S_all.append(S.reshape(NCOL * 128, 128))

        nf = np.zeros((NODES_PAD, DIN), ml_dtypes.bfloat16)
        nf[: int(n_count[core])] = node_feat[n_lo[core]:n_hi[core]]
        nf_all.append(nf)

        # pooling blocks
        glocal = graph_id[n_lo[core]:n_hi[core]] - g_lo[core]
        Pm = np.zeros((NPB, 128, 128), ml_dtypes.bfloat16)
        for b, (gt, t) in enumerate(pool_blocks):
            lo_n = t * P
            hi_n = min((t + 1) * P, int(n_count[core]))
            if hi_n <= lo_n:
                continue
            gl = glocal[lo_n:hi_n]
            in_gt = (gl >= gt * P) & (gl < (gt + 1) * P)
            rows = np.nonzero(in_gt)[0]
            Pm[b, rows, gl[rows] - gt * P] = 1.0
        P_all.append(Pm.reshape(NPB * 128, 128))

    return dict(
        N=N, G=G, NODES_PAD=NODES_PAD, NT=NT, TABLE_ROWS=TABLE_ROWS,
        NCHUNK=NCHUNK, NF_TOTAL=NF_TOTAL, G_PAD=G_PAD, GT=GT, NCOL=NCOL,
        groups=groups, pool_blocks=pool_blocks, pool_by_gt=pool_by_gt, NPB=NPB,
        S_all=S_all, idx_all=idx_all, nf_all=nf_all, P_all=P_all,
        g_lo=g_lo, g_hi=g_hi, G_CORE=G_CORE,
    )


def _build(meta):
    NODES_PAD = meta["NODES_PAD"]
    NT = meta["NT"]
    NCHUNK = meta["NCHUNK"]
    TABLE_ROWS = meta["TABLE_ROWS"]
    NCOL = meta["NCOL"]
    NPB = meta["NPB"]
    G_PAD = meta["G_PAD"]
    GT = meta["GT"]
    N_GLOBAL = meta["N"]
    NF = float(meta["NF_TOTAL"])
    groups = meta["groups"]
    pool_by_gt = meta["pool_by_gt"]

    nc = bacc.Bacc("TRN2", target_bir_lowering=False, debug=False,
                   num_devices=NCORES, num_swdge_queues=4)

    # ---- I/O ----
    nf_in = nc.dram_tensor("nf", [NODES_PAD, DIN], BF16, kind="ExternalInput")
    gidx_in = nc.dram_tensor("gidx", [128, NCOL * 8], I16, kind="ExternalInput")
    S_in = nc.dram_tensor("S", [NCOL * 128, 128], BF16, kind="ExternalInput")
    P_in = nc.dram_tensor("Pm", [NPB * 128, 128], BF16, kind="ExternalInput")
    wemb_in = nc.dram_tensor("wemb", [DIN, H], BF16, kind="ExternalInput")
    bemb_in = nc.dram_tensor("bemb", [H, 1], F32, kind="ExternalInput")
    wc_in = nc.dram_tensor("wc", [L * H, H], BF16, kind="ExternalInput")
    wr_in = nc.dram_tensor("wr", [L * H, H], BF16, kind="ExternalInput")
    cb_in = nc.dram_tensor("cb", [L * H, 1], F32, kind="ExternalInput")
    rb_in = nc.dram_tensor("rb", [L * H, 1], F32, kind="ExternalInput")
    bng_in = nc.dram_tensor("bng", [L * H, 1], F32, kind="ExternalInput")
    bnb_in = nc.dram_tensor("bnb", [L * H, 1], F32, kind="ExternalInput")
    w1_in = nc.dram_tensor("w1", [H, 1024], BF16, kind="ExternalInput")
    b1_in = nc.dram_tensor("b1", [1024, 1], F32, kind="ExternalInput")
    w2_in = nc.dram_tensor("w2", [1024, 1], BF16, kind="ExternalInput")
    b2_in = nc.dram_tensor("b2", [1, 1], F32, kind="ExternalInput")
    out_dram = nc.dram_tensor("out", [1, G_PAD], F32, kind="ExternalOutput")

    y_bounce = [nc.dram_tensor(f"yb{l}", [NODES_PAD, HP], BF16) for l in range(L)]
    y_full = [nc.dram_tensor(f"yf{l}", [TABLE_ROWS, HP], BF16, addr_space="Shared")
              for l in range(L)]
    st_in = [nc.dram_tensor(f"sti{l}", [2 * 128, 2], F32) for l in range(L)]
    st_out = [nc.dram_tensor(f"sto{l}", [2 * 128, 2], F32, addr_space="Shared")
              for l in range(L)]
    RG = [list(range(NCORES))]

    max_gcols = max(max(nc_ for _, _, nc_ in g["c_slices"]) for g in groups)
    max_tgcols = max(g["ncols"] for g in groups)
    NJ = NODES_PAD // 512  # 512-col chunks

    with tile.TileContext(nc) as tc:
        nc.gpsimd.load_library(_mlp_lib)
        with (
            tc.tile_pool(name="state", bufs=1) as stp,
            tc.tile_pool(name="const", bufs=1) as cst,
            tc.tile_pool(name="work", bufs=3) as wk,
            tc.tile_pool(name="gath", bufs=2) as gp,
            tc.tile_pool(name="spool", bufs=2) as sp,
            tc.tile_pool(name="psy", bufs=2, space="PSUM") as psy,
            tc.tile_pool(name="psagg", bufs=2, space="PSUM") as psagg,
            tc.tile_pool(name="pst", bufs=2, space="PSUM") as pst,
            tc.tile_pool(name="psr", bufs=2, space="PSUM") as psr,
        ):
            ident = cst.tile([128, 128], BF16, tag="ident")
            make_identity(nc, ident[:])

            # ---- load weights ----
            wemb = cst.tile([DIN, H], BF16, tag="wemb")
            nc.sync.dma_start(out=wemb[:], in_=wemb_in[:])
            bemb = []
            for m, (f0, fw) in enumerate(FCH):
                t_ = cst.tile([fw, 1], F32, tag=f"bemb{m}", name=f"bemb{m}")
                nc.sync.dma_start(out=t_[:], in_=bemb_in[f0:f0 + fw, :])
                bemb.append(t_)
            w1 = []
            for k, (f0, fw) in enumerate(FCH):
                t_ = cst.tile([fw, 1024], BF16, tag=f"w1_{k}", name=f"w1_{k}")
                nc.sync.dma_start(out=t_[:], in_=w1_in[f0:f0 + fw, :])
                w1.append(t_)
            b1 = cst.tile([128, 8], F32, tag="b1")
            nc.sync.dma_start(out=b1[:], in_=b1_in.ap().rearrange("(a p) one -> p (a one)", p=128))
            w2 = cst.tile([128, 8], BF16, tag="w2")
            nc.sync.dma_start(out=w2[:], in_=w2_in.ap().rearrange("(a p) one -> p (a one)", p=128))
            b2 = cst.tile([1, 1], F32, tag="b2")
            nc.sync.dma_start(out=b2[:], in_=b2_in[:])

            def per_layer_consts(l):
                out = {}
                for nm, src_t in (("wc", wc_in), ("wr", wr_in)):
                    chunks = []
                    for k, (f0, fw) in enumerate(FCH):
                        t_ = wk.tile([fw, H], BF16, tag=f"{nm}k{k}", name=f"{nm}k{k}")
                        nc.sync.dma_start(out=t_[:], in_=src_t[l * H + f0: l * H + f0 + fw, :])
                        chunks.append(t_)
                    out[nm] = chunks
                for nm, src_t in (("cb", cb_in), ("rb", rb_in), ("bng", bng_in), ("bnb", bnb_in)):
                    chunks = []
                    for m, (f0, fw) in enumerate(FCH):
                        t_ = wk.tile([fw, 1], F32, tag=f"{nm}m{m}", name=f"{nm}m{m}")
                        nc.sync.dma_start(out=t_[:], in_=src_t[l * H + f0: l * H + f0 + fw, :])
                        chunks.append(t_)
                    out[nm] = chunks
                return out

            # ---- state ----
            xT = [stp.tile([128, NODES_PAD], BF16, tag="xA0", name="xA0"),
                  stp.tile([72, NODES_PAD], BF16, tag="xA1", name="xA1")]
            z = [stp.tile([128, 1], BF16, tag="z0", name="z0"),
                 stp.tile([72, 1], BF16, tag="z1", name="z1")]

            # ---- embedding: xT = (node_feat @ emb_W + emb_b)^T ----
            for j in range(NJ):
                nfT = wk.tile([DIN, 512], BF16, tag="nfT")
                for q in range(4):
                    nt = j * 4 + q
                    nft = wk.tile([128, DIN], BF16, tag="nft")
                    nc.sync.dma_start(out=nft[:], in_=nf_in[nt * 128:(nt + 1) * 128, :])
                    ptt = pst.tile([128, 128], BF16, tag="pt")
                    nc.tensor.transpose(out=ptt[:DIN, :], in_=nft[:], identity=ident[:])
                    nc.vector.tensor_copy(out=nfT[:, q * 128:(q + 1) * 128], in_=ptt[:DIN, :])
                for m, (f0, fw) in enumerate(FCH):
                    ps = psr.tile([128, 512], F32, tag="pr")
                    nc.tensor.matmul(ps[:fw, :], lhsT=wemb[:, f0:f0 + fw], rhs=nfT[:],
                                     start=True, stop=True)
                    nc.scalar.activation(out=xT[m][:, j * 512:(j + 1) * 512], in_=ps[:fw, :],
                                         func=AF.Identity, bias=bemb[m][:], scale=1.0)
            for m in range(2):
                nc.vector.tensor_copy(out=z[m][:], in_=bemb[m][:])

            # ---- layers ----
            NL = int(os.environ.get("GCN_LAYERS", str(L)))
            SKIP_GATHER = os.environ.get("GCN_NO_GATHER", "0") == "1"
            SKIP_CC = os.environ.get("GCN_NO_CC", "0") == "1"
            STAGE = int(os.environ.get("GCN_STAGE", "9"))
            for l in range(NL):
                C = per_layer_consts(l)
                # y = x @ Wc (node-major), write to DRAM
                for nt in range(NT):
                    ps = psy.tile([128, H], F32, tag="py")
                    for k, (f0, fw) in enumerate(FCH):
                        nc.tensor.matmul(ps[:], lhsT=xT[k][:, nt * 128:(nt + 1) * 128],
                                         rhs=C["wc"][k][:], start=(k == 0), stop=(k == 1))
                    yt = wk.tile([128, HP], BF16, tag="yt")
                    nc.scalar.activation(out=yt[:, :H], in_=ps[:], func=AF.Copy, bias=0.0, scale=1.0)
                    nc.vector.memset(yt[:, H:], 0.0)
                    nc.sync.dma_start(out=y_bounce[l][nt * 128:(nt + 1) * 128, :], in_=yt[:])

                if not SKIP_CC:
                    nc.gpsimd.collective_compute(
                        "AllGather", ALU.bypass, replica_groups=RG,
                        ins=[y_bounce[l].ap().opt()], outs=[y_full[l].ap().opt()])

                # residual: x <- relu(x @ Wr + br)^T in place (y already used x)
                for j in range(NJ if STAGE >= 2 else 0):
                    pss = []
                    for m, (f0, fw) in enumerate(FCH):
                        psm = psr.tile([128, 512], F32, tag="pr", name=f"psm{m}")
                        for k, (kf0, kfw) in enumerate(FCH):
                            nc.tensor.matmul(psm[:fw, :], lhsT=C["wr"][k][:, f0:f0 + fw],
                                             rhs=xT[k][:, j * 512:(j + 1) * 512],
                                             start=(k == 0), stop=(k == 1))
                        pss.append(psm)
                    for m, (f0, fw) in enumerate(FCH):
                        nc.scalar.activation(out=xT[m][:, j * 512:(j + 1) * 512],
                                             in_=pss[m][:fw, :],
                                             func=AF.Relu, bias=C["rb"][m][:], scale=1.0)

                # gather + S-matmul + transpose + relu
                qrr = 0
                for gi, g in enumerate(groups if not SKIP_GATHER else []):
                    ncols_tg = g["ncols"]
                    gt_tile = gp.tile([128, max_tgcols, HP], BF16, tag="gt")
                    idxt = wk.tile([128, max_tgcols * 8], I16, tag="idx")
                    nc.sync.dma_start(out=idxt[:, :ncols_tg * 8],
                                      in_=gidx_in[:, g["col0"] * 8:(g["col0"] + ncols_tg) * 8])
                    S_tg = sp.tile([128, max_tgcols, 128], BF16, tag="stg")
                    nc.sync.dma_start(
                        out=S_tg[:, :ncols_tg, :],
                        in_=S_in[g["col0"] * 128:(g["col0"] + ncols_tg) * 128, :]
                        .rearrange("(b p) m -> p b m", p=128))
                    for (c, c0, ncols) in g["c_slices"]:
                        if ncols == 0:
                            continue
                        pos0 = c0 - g["col0"]
                        rows_c = min(CHUNK, TABLE_ROWS - c * CHUNK)
                        nc.gpsimd.dma_gather(
                            gt_tile[:, pos0:pos0 + ncols, :],
                            y_full[l][c * CHUNK:c * CHUNK + rows_c, :],
                            idxt[:, pos0 * 8:(pos0 + ncols) * 8],
                            ncols * 128, ncols * 128, HP,
                            single_packet=False, queue_num=qrr % 4)
                        qrr += 1
                    for t, blocks in g["tile_blocks"].items():
                        ps = psagg.tile([128, H], F32, tag="pa")
                        nb = len(blocks)
                        for bi, colg in enumerate(blocks):
                            pos = colg - g["col0"]
                            nc.tensor.matmul(
                                ps[:], lhsT=S_tg[:, pos, :], rhs=gt_tile[:, pos, :H],
                                start=(bi == 0), stop=(bi == nb - 1))
                        a_sb = wk.tile([128, H], BF16, tag="asb")
                        nc.vector.tensor_copy(out=a_sb[:], in_=ps[:])
                        for m, (f0, fw) in enumerate(FCH):
                            ptt = pst.tile([128, 128], BF16, tag="pt")
                            nc.tensor.transpose(out=ptt[:fw, :], in_=a_sb[:, f0:f0 + fw],
                                                identity=ident[:])
                            ht = wk.tile([128, 128], BF16, tag="ht", bufs=2)
                            nc.scalar.activation(
                                out=ht[:fw, :], in_=ptt[:fw, :],
                                func=AF.Relu, bias=C["cb"][m][:], scale=1.0)
                            nc.vector.tensor_add(
                                out=xT[m][:, t * 128:(t + 1) * 128],
                                in0=xT[m][:, t * 128:(t + 1) * 128], in1=ht[:fw, :])

                # ghost column: z_h = relu(bc) + relu(Wr^T z + br)
                zh = []
                for m, (f0, fw) in enumerate(FCH if STAGE >= 3 else []):
                    ps = psy.tile([128, H], F32, tag="py")
                    for k, (kf0, kfw) in enumerate(FCH):
                        nc.tensor.matmul(ps[:fw, :1], lhsT=C["wr"][k][:, f0:f0 + fw],
                                         rhs=z[k][:], start=(k == 0), stop=(k == 1))
                    zr = wk.tile([128, 1], BF16, tag=f"zr{m}")
                    nc.scalar.activation(out=zr[:fw, :], in_=ps[:fw, :1], func=AF.Relu,
                                         bias=C["rb"][m][:], scale=1.0)
                    zc = wk.tile([128, 1], BF16, tag=f"zc{m}")
                    nc.scalar.activation(out=zc[:fw, :], in_=C["cb"][m][:], func=AF.Relu,
                                         bias=0.0, scale=1.0)
                    zt = wk.tile([128, 1], BF16, tag=f"zh{m}")
                    nc.vector.tensor_add(out=zt[:fw, :], in0=zr[:fw, :], in1=zc[:fw, :])
                    zh.append(zt)

                # BN stats
                sacc = [wk.tile([128, NJ], F32, tag=f"sacc{m}", name=f"sacc{m}") for m in range(2)]
                qacc = [wk.tile([128, NJ], F32, tag=f"qacc{m}", name=f"qacc{m}") for m in range(2)]
                for j in range(NJ if STAGE >= 4 else 0):
                    for m, (f0, fw) in enumerate(FCH):
                        hsl = xT[m][:, j * 512:(j + 1) * 512]
                        nc.scalar.activation(out=hsl, in_=hsl, func=AF.Copy, bias=0.0,
                                             scale=1.0, accum_out=sacc[m][:fw, j:j + 1])
                        scr = wk.tile([128, 512], BF16, tag="scr", bufs=2)
                        nc.scalar.activation(out=scr[:fw, :], in_=hsl, func=AF.Square,
                                             bias=0.0, scale=1.0,
                                             accum_out=qacc[m][:fw, j:j + 1])
                stat = [wk.tile([128, 2], F32, tag=f"stat{m}", name=f"stat{m}") for m in range(2)]
                for m, (f0, fw) in enumerate(FCH if STAGE >= 4 else []):
                    nc.scalar.activation(out=sacc[m][:fw, :], in_=sacc[m][:fw, :],
                                         func=AF.Copy, bias=0.0, scale=1.0,
                                         accum_out=stat[m][:fw, 0:1])
                    nc.scalar.activation(out=qacc[m][:fw, :], in_=qacc[m][:fw, :],
                                         func=AF.Copy, bias=0.0, scale=1.0,
                                         accum_out=stat[m][:fw, 1:2])
                    nc.sync.dma_start(out=st_in[l][m * 128:m * 128 + fw, :], in_=stat[m][:fw, :])
                if STAGE >= 4:
                    if not SKIP_CC:
                        nc.gpsimd.collective_compute(
                            "AllReduce", ALU.add, replica_groups=RG,
                            ins=[st_in[l].ap().opt()], outs=[st_out[l].ap().opt()])
                    else:
                        nc.sync.dma_start(out=st_out[l][:], in_=st_in[l][:])

                # finalize scale/shift per feature chunk
                scale = [wk.tile([128, 1], F32, tag=f"scale{m}", name=f"scale{m}") for m in range(2)]
                shift = [wk.tile([128, 1], F32, tag=f"shift{m}", name=f"shift{m}") for m in range(2)]
                for m, (f0, fw) in enumerate(FCH if STAGE >= 5 else []):
                    st = wk.tile([128, 2], F32, tag=f"sin{m}")
                    nc.sync.dma_start(out=st[:fw, :], in_=st_out[l][m * 128:m * 128 + fw, :])
                    zf = wk.tile([128, 1], F32, tag=f"zf{m}")
                    nc.vector.tensor_copy(out=zf[:fw, :], in_=zh[m][:fw, :])
                    t1 = wk.tile([128, 6], F32, tag=f"tmp{m}")
                    # t1[:,0] = sum_real; t1[:,1] = mean
                    nc.vector.tensor_scalar(out=t1[:fw, 0:1], in0=zf[:fw, :], scalar1=NF,
                                            scalar2=None, op0=ALU.mult)
                    nc.vector.tensor_tensor(out=t1[:fw, 0:1], in0=st[:fw, 0:1],
                                            in1=t1[:fw, 0:1], op=ALU.subtract)
                    nc.vector.tensor_scalar(out=t1[:fw, 1:2], in0=t1[:fw, 0:1],
                                            scalar1=1.0 / N_GLOBAL, scalar2=None, op0=ALU.mult)
                    # t1[:,2] = sumsq_real/N ; t1[:,3] = var + eps
                    nc.vector.tensor_tensor(out=t1[:fw, 2:3], in0=zf[:fw, :], in1=zf[:fw, :],
                                            op=ALU.mult)
                    nc.vector.tensor_scalar(out=t1[:fw, 2:3], in0=t1[:fw, 2:3], scalar1=NF,
                                            scalar2=None, op0=ALU.mult)
                    nc.vector.tensor_tensor(out=t1[:fw, 2:3], in0=st[:fw, 1:2],
                                            in1=t1[:fw, 2:3], op=ALU.subtract)
                    nc.vector.tensor_scalar(out=t1[:fw, 2:3], in0=t1[:fw, 2:3],
                                            scalar1=1.0 / N_GLOBAL, scalar2=None, op0=ALU.mult)
                    nc.vector.tensor_tensor(out=t1[:fw, 3:4], in0=t1[:fw, 1:2],
                                            in1=t1[:fw, 1:2], op=ALU.mult)
                    nc.vector.tensor_tensor(out=t1[:fw, 3:4], in0=t1[:fw, 2:3],
                                            in1=t1[:fw, 3:4], op=ALU.subtract)
                    nc.vector.tensor_scalar(out=t1[:fw, 3:4], in0=t1[:fw, 3:4], scalar1=EPS,
                                            scalar2=None, op0=ALU.add)
                    # t1[:,4] = sqrt(var+eps); t1[:,5] = 1/sqrt
                    nc.scalar.activation(out=t1[:fw, 4:5], in_=t1[:fw, 3:4], func=AF.Sqrt,
                                         bias=0.0, scale=1.0)
                    nc.vector.reciprocal(out=t1[:fw, 5:6], in_=t1[:fw, 4:5])
                    nc.vector.tensor_tensor(out=scale[m][:fw, :], in0=C["bng"][m][:],
                                            in1=t1[:fw, 5:6], op=ALU.mult)
                    nc.vector.tensor_tensor(out=shift[m][:fw, :], in0=t1[:fw, 1:2],
                                            in1=scale[m][:fw, :], op=ALU.mult)
                    nc.vector.tensor_tensor(out=shift[m][:fw, :], in0=C["bnb"][m][:],
                                            in1=shift[m][:fw, :], op=ALU.subtract)
                # apply BN in place on xT
                for j in range(NJ if STAGE >= 5 else 0):
                    for m, (f0, fw) in enumerate(FCH):
                        hsl = xT[m][:, j * 512:(j + 1) * 512]
                        nc.vector.tensor_scalar(out=hsl, in0=hsl, scalar1=scale[m][:fw, :],
                                                scalar2=shift[m][:fw, :],
                                                op0=ALU.mult, op1=ALU.add)
                for m, (f0, fw) in enumerate(FCH if STAGE >= 5 else []):
                    nc.vector.tensor_scalar(out=z[m][:fw, :], in0=zh[m][:fw, :],
                                            scalar1=scale[m][:fw, :], scalar2=shift[m][:fw, :],
                                            op0=ALU.mult, op1=ALU.add)

            # ---- pooling ----
            pooledT = [stp.tile([128, G_PAD], BF16, tag="plT0", name="plT0"),
                       stp.tile([72, G_PAD], BF16, tag="plT1", name="plT1")]
            for gt in range(GT):
                blocks = pool_by_gt[gt]
                pp = psy.tile([128, H], F32, tag="py")
                for bi, (b, t) in enumerate(blocks):
                    xnm = wk.tile([128, H], BF16, tag="xnm")
                    for m, (f0, fw) in enumerate(FCH):
                        ptt = pst.tile([128, 128], BF16, tag="pt")
                        nc.tensor.transpose(out=ptt[:128, :fw],
                                            in_=xT[m][:, t * 128:(t + 1) * 128],
                                            identity=ident[:fw, :fw])
                        nc.vector.tensor_copy(out=xnm[:, f0:f0 + fw], in_=ptt[:128, :fw])
                    Pb = sp.tile([128, 128], BF16, tag="pb")
                    nc.sync.dma_start(out=Pb[:], in_=P_in[b * 128:(b + 1) * 128, :])
                    nc.tensor.matmul(pp[:], lhsT=Pb[:], rhs=xnm[:],
                                     start=(bi == 0), stop=(bi == len(blocks) - 1))
                pl_sb = wk.tile([128, H], BF16, tag="plsb")
                nc.vector.tensor_copy(out=pl_sb[:], in_=pp[:])
                for m, (f0, fw) in enumerate(FCH):
                    ptt = pst.tile([128, 128], BF16, tag="pt")
                    nc.tensor.transpose(out=ptt[:fw, :], in_=pl_sb[:, f0:f0 + fw],
                                        identity=ident[:])
                    nc.vector.tensor_copy(out=pooledT[m][:, gt * 128:(gt + 1) * 128],
                                          in_=ptt[:fw, :])

            # ---- MLP ----
            NGC = _ceil(G_PAD, 128)
            for gc in range(NGC):
                gw = 128
                ps2 = psagg.tile([128, H], F32, tag="pa")
                for m8 in range(8):
                    ps = psr.tile([128, 512], F32, tag="pr")
                    for k, (f0, fw) in enumerate(FCH):
                        nc.tensor.matmul(ps[:, :gw], lhsT=w1[k][:, m8 * 128:(m8 + 1) * 128],
                                         rhs=pooledT[k][:, gc * 128:gc * 128 + gw],
                                         start=(k == 0), stop=(k == 1))
                    h1t = wk.tile([128, 512], BF16, tag="h1t", bufs=2)
                    nc.scalar.activation(out=h1t[:, :gw],
                                         in_=ps[:, :gw], func=AF.Relu,
                                         bias=b1[:, m8:m8 + 1], scale=1.0)
                    nc.tensor.matmul(ps2[:1, :gw], lhsT=w2[:, m8:m8 + 1],
                                     rhs=h1t[:, :gw],
                                     start=(m8 == 0), stop=(m8 == 7))
                o_sb = wk.tile([1, 512], F32, tag="osb", bufs=1)
                nc.vector.tensor_scalar(out=o_sb[:1, :gw], in0=ps2[:1, :gw],
                                        scalar1=b2[:1, :], scalar2=None, op0=ALU.add)
                nc.sync.dma_start(out=out_dram[0:1, gc * 128:gc * 128 + gw],
                                  in_=o_sb[:1, :gw])

    nc.compile()
    return nc


def kernel(node_feat, src, dst, graph_id, emb_W, emb_b, conv_W, conv_b,
           res_W, res_b, bn_g, bn_b, out_W1, out_b1, out_W2, out_b2):
    node_feat = np.asarray(node_feat, dtype=np.float32)
    G = 4096
    meta = _preprocess(node_feat, src, dst, graph_id, G=G)

    bf = ml_dtypes.bfloat16
    wemb = np.ascontiguousarray(np.asarray(emb_W, np.float32).astype(bf))
    wc = np.ascontiguousarray(np.asarray(conv_W, np.float32).reshape(L * H, H).astype(bf))
    wr = np.ascontiguousarray(np.asarray(res_W, np.float32).reshape(L * H, H).astype(bf))
    w1 = np.ascontiguousarray(np.asarray(out_W1, np.float32).astype(bf))
    w2 = np.ascontiguousarray(np.asarray(out_W2, np.float32).astype(bf))
    bemb = np.asarray(emb_b, np.float32).reshape(H, 1)
    cb = np.asarray(conv_b, np.float32).reshape(L * H, 1)
    rb = np.asarray(res_b, np.float32).reshape(L * H, 1)
    bng = np.asarray(bn_g, np.float32).reshape(L * H, 1)
    bnb = np.asarray(bn_b, np.float32).reshape(L * H, 1)
    b1 = np.asarray(out_b1, np.float32).reshape(1024, 1)
    b2 = np.asarray(out_b2, np.float32).reshape(1, 1)

    nc = _build(meta)

    global _last_nc, _last_in_maps
    in_maps = []
    for c in range(NCORES):
        in_maps.append({
            "nf": meta["nf_all"][c], "gidx": meta["idx_all"][c],
            "S": meta["S_all"][c], "Pm": meta["P_all"][c],
            "wemb": wemb, "bemb": bemb, "wc": wc, "wr": wr,
            "cb": cb, "rb": rb, "bng": bng, "bnb": bnb,
            "w1": w1, "b1": b1, "w2": w2, "b2": b2,
        })
    _last_nc, _last_in_maps = nc, in_maps
    res = run_bass_kernel_spmd(nc, in_maps, core_ids=list(range(NCORES)))

    out = np.zeros((G, 1), np.float32)
    for c in range(NCORES):
        gc_ = int(meta["G_CORE"][c])
        vals = res.results[c]["out"][0, :gc_]
        out[meta["g_lo"][c]:meta["g_hi"][c], 0] = vals
    return out



# revision 4
# speedup vs baseline: 4.3690x; 4.3690x over previous
"""GCN (nn_GCNModel) Trainium2 kernel — 8 NeuronCores, SPMD.

Design:
  - Shard nodes/edges by graph (graph_id sorted -> contiguous node ranges per core).
  - Feature-major on-chip state: xT [H(2 part-chunks 128+72), NODES_PAD] bf16.
  - Per layer: y = x @ conv_W computed locally (node-major tiles), written to DRAM,
    AllGather -> full y table [8*NODES_PAD, 256] bf16 (rows padded to 512B).
  - Edge aggregation: dma_gather (int16 idx, 32768-row chunks, 4 SWDGE queues)
    pulls y[src] rows for local edges (sorted by (tile-group, chunk, dst-tile)),
    then a 0/1 S-matrix matmul on the tensor engine segment-sums them per
    128-node dst tile: agg = S^T @ gathered  (PSUM fp32).
  - h = relu(agg + conv_b) (transposed to feature-major) + relu(x @ res_W + res_b).
  - BatchNorm over all nodes: per-partition (feature) sums via ACT accum_out /
    tensor_tensor_reduce, AllReduce of [sum, sumsq], ghost-column correction for
    padded fake nodes, then x = h*scale + shift in place.
  - Pooling: per-graph segment-sum via 0/1 P-matrix matmuls; readout MLP on PE.

Transport optimization (per-call input bytes dominate the measured time at
~0.55 ms/MB/core): the one-hot S and P matrices are never shipped — only
1-byte-per-slot dst/graph maps, expanded on device via iota+is_equal into an
internal DRAM S table (once) and SBUF P blocks (at pooling). Gather indices
ship unreplicated [16, NCOL*8] and are expanded 8x into a resident SBUF tile.
Weights are identical across cores and baked into the NEFF as Const tensors.
"""
import math
import os
import numpy as np
import ml_dtypes

import concourse.bass as bass
import concourse.bacc as bacc
import concourse.mybir as mybir
import concourse.tile as tile
from concourse.library_config import mlp as _mlp_lib
from concourse.masks import make_identity
from concourse.bass_utils import run_bass_kernel_spmd

BF16 = mybir.dt.bfloat16
F32 = mybir.dt.float32
I16 = mybir.dt.int16
I8 = mybir.dt.int8

NCORES = 8
P = 128
H = 200
DIN = 74
L = 5
HP = 256          # padded feature row (bf16 -> 512B, %256B for dma_gather)
CHUNK = 32768     # int16 index range per gather chunk
TG = 2            # node-tiles per gather group
EPS = 1e-5
FCH = [(0, 128), (128, 72)]   # feature chunks (offset, width)


# --- patch: partition DMASW sem lanes by SWDGE queue so multi-queue dma_gather
# keeps each DMA-completion semaphore locked to a single queue (Tile's default
# round-robin assigns lanes in scheduled order, which mixes queues on a lane).
import concourse.tile_sem_assignment as _tsa
import concourse.bass_isa as _bass_isa

if not getattr(_tsa.TileClockTick, "_gcn_queue_patch", False):
    _orig_assign_tick = _tsa.TileClockTick._assign_tick

    def _patched_assign_tick(self, inst):
        if (isinstance(inst, _tsa.DMAInst)
                and not isinstance(inst, _bass_isa.UserSyncedRemoteDMADescs)
                and inst.engine == mybir.EngineType.Pool):
            q = int(getattr(inst, "queue_num", 0) or 0) % 4
            if not hasattr(self, "_gcn_qtgl"):
                self._gcn_qtgl = {}
            tgl = self._gcn_qtgl.get(q, 0)
            self._gcn_qtgl[q] = tgl ^ 1
            self.next_sw_dma_idx = (q + 4 * tgl) % self.swdge_sem_count
        return _orig_assign_tick(self, inst)

    _tsa.TileClockTick._assign_tick = _patched_assign_tick
    _tsa.TileClockTick._gcn_queue_patch = True

AF = mybir.ActivationFunctionType
ALU = mybir.AluOpType


def _ceil(a, b):
    return -(-a // b)


def _preprocess(node_feat, src, dst, graph_id, **kw):
    N = node_feat.shape[0]
    G = int(kw["G"])
    src = np.asarray(src).astype(np.int64)
    dst = np.asarray(dst).astype(np.int64)
    graph_id = np.asarray(graph_id).astype(np.int64)

    counts = np.bincount(graph_id, minlength=G)
    gstart = np.zeros(G + 1, np.int64)
    gstart[1:] = np.cumsum(counts)

    # graph-boundary cuts closest to uniform node split
    g_cuts = [0]
    for c in range(1, NCORES):
        target = round(c * N / NCORES)
        g = int(np.searchsorted(gstart, target))
        g = min(g, G)
        if g > 0 and abs(int(gstart[g - 1]) - target) <= abs(int(gstart[g]) - target):
            g -= 1
        g = max(g, g_cuts[-1])
        g_cuts.append(g)
    g_cuts.append(G)
    g_lo = np.array(g_cuts[:-1])
    g_hi = np.array(g_cuts[1:])
    n_lo = gstart[g_lo]
    n_hi = gstart[g_hi]
    n_count = n_hi - n_lo

    NODES_PAD = _ceil(int(n_count.max()), 512) * 512
    NT = NODES_PAD // P
    TABLE_ROWS = NCORES * NODES_PAD
    NCHUNK = _ceil(TABLE_ROWS, CHUNK)
    NF_TOTAL = TABLE_ROWS - N  # global fake-node count

    G_CORE = g_hi - g_lo
    G_PAD = _ceil(int(G_CORE.max()), P) * P
    GT = G_PAD // P

    # --- edges ---
    bounds = n_lo  # n_lo[0] == 0
    src_core = np.searchsorted(bounds, src, side="right") - 1
    dst_core = np.searchsorted(bounds, dst, side="right") - 1
    table_row = src_core * NODES_PAD + (src - n_lo[src_core])
    e_chunk = table_row // CHUNK
    e_local = table_row % CHUNK

    # per-core per-(chunk,tile) counts -> uniform K grid
    ecnt = np.zeros((NCORES, NCHUNK, NT), np.int64)
    dst_local_all = dst - n_lo[dst_core]
    t_all = dst_local_all // P
    np.add.at(ecnt, (dst_core, e_chunk, t_all), 1)
    K = _ceil(ecnt.max(axis=0), 128)
    # ensure every tile has >=1 block so its psum gets written
    empty_t = K.sum(axis=0) == 0
    K[0][empty_t] = 1

    # column enumeration: (tile-group, chunk, tile, k)
    colbase = np.full((NCHUNK, NT), -1, np.int64)
    groups = []
    col = 0
    for tg0 in range(0, NT, TG):
        tg_tiles = list(range(tg0, min(tg0 + TG, NT)))
        tg_col0 = col
        c_slices = []
        tile_blocks = {t: [] for t in tg_tiles}
        for c in range(NCHUNK):
            c0 = col
            for t in tg_tiles:
                colbase[c, t] = col if K[c, t] > 0 else -1
                for k in range(int(K[c, t])):
                    tile_blocks[t].append(col)
                    col += 1
            c_slices.append((c, c0, col - c0))
        groups.append(dict(col0=tg_col0, ncols=col - tg_col0,
                           c_slices=c_slices, tile_blocks=tile_blocks))
    NCOL = col

    # --- per-core dmap + gather idx ---
    dmap_all, idx_all, nf_all, gmap_all = [], [], [], []

    # pooling block grid (union over cores)
    t_lo_g = np.full(GT, 10 ** 9, np.int64)
    t_hi_g = np.full(GT, -1, np.int64)
    gsl_local = []
    for core in range(NCORES):
        gsl = gstart[g_lo[core]:g_hi[core] + 1] - n_lo[core]  # local graph starts
        gsl_local.append(gsl)
        for gt in range(GT):
            glo = gt * P
            ghi = min((gt + 1) * P, int(G_CORE[core]))
            if glo >= ghi:
                continue
            lo = int(gsl[glo])
            hi = int(gsl[ghi])
            if hi <= lo:
                continue
            t_lo_g[gt] = min(t_lo_g[gt], lo // P)
            t_hi_g[gt] = max(t_hi_g[gt], _ceil(hi, P))
    pool_blocks = []  # list of (gt, t)
    for gt in range(GT):
        if t_hi_g[gt] < 0:
            pool_blocks.append((gt, 0))
        else:
            for t in range(int(t_lo_g[gt]), int(t_hi_g[gt])):
                pool_blocks.append((gt, t))
    NPB = len(pool_blocks)
    pool_by_gt = {}
    for b, (gt, t) in enumerate(pool_blocks):
        pool_by_gt.setdefault(gt, []).append((b, t))

    for core in range(NCORES):
        sel = dst_core == core
        e_c = e_chunk[sel]
        e_l = e_local[sel]
        d_loc = dst[sel] - n_lo[core]
        t_e = d_loc // P
        tg_e = t_e // TG
        order = np.lexsort((d_loc, t_e, e_c, tg_e))
        e_c, e_l, d_loc, t_e = e_c[order], e_l[order], d_loc[order], t_e[order]

        # rank within (chunk, tile) segment
        seg_key = e_c * NT + t_e
        # edges are sorted so equal seg_keys are consecutive within a tg, and
        # (c,t) pairs don't repeat across tgs (t determines tg)
        change = np.empty(len(seg_key), bool)
        if len(seg_key):
            change[0] = True
            change[1:] = seg_key[1:] != seg_key[:-1]
        seg_start = np.maximum.accumulate(np.where(change, np.arange(len(seg_key)), 0))
        rank = np.arange(len(seg_key)) - seg_start

        col_e = colbase[e_c, t_e] + rank // 128
        r_e = rank % 128

        idx_flat = np.zeros(NCOL * 128, np.int16)
        idx_flat[col_e * 128 + r_e] = e_l.astype(np.int16)
        # wrap: position q -> [q%16, q//16]; device replicates across 8 groups
        wrapped = idx_flat.reshape(NCOL * 8, 16).T  # [16, NCOL*8]
        idx_all.append(np.ascontiguousarray(wrapped))

        # dst map: slot r of column col -> dst offset within tile (255 = empty)
        dm = np.full((128, NCOL), 255, np.int16)
        dm[r_e, col_e] = (d_loc - t_e * P).astype(np.int16)
        dmap_all.append(dm.astype(ml_dtypes.bfloat16))

        nf = np.zeros((NODES_PAD, DIN), ml_dtypes.bfloat16)
        nf[: int(n_count[core])] = node_feat[n_lo[core]:n_hi[core]]
        nf_all.append(nf)

        # pooling map: node slot r of tile t (block b=(gt,t)) -> graph offset
        glocal = graph_id[n_lo[core]:n_hi[core]] - g_lo[core]
        gm = np.full((128, NPB), 255, np.int16)
        for b, (gt, t) in enumerate(pool_blocks):
            lo_n = t * P
            hi_n = min((t + 1) * P, int(n_count[core]))
            if hi_n <= lo_n:
                continue
            gl = glocal[lo_n:hi_n]
            in_gt = (gl >= gt * P) & (gl < (gt + 1) * P)
            rows = np.nonzero(in_gt)[0]
            gm[rows, b] = (gl[rows] - gt * P).astype(np.int16)
        gmap_all.append(gm.astype(ml_dtypes.bfloat16))

    return dict(
        N=N, G=G, NODES_PAD=NODES_PAD, NT=NT, TABLE_ROWS=TABLE_ROWS,
        NCHUNK=NCHUNK, NF_TOTAL=NF_TOTAL, G_PAD=G_PAD, GT=GT, NCOL=NCOL,
        groups=groups, pool_blocks=pool_blocks, pool_by_gt=pool_by_gt, NPB=NPB,
        dmap_all=dmap_all, idx_all=idx_all, nf_all=nf_all, gmap_all=gmap_all,
        g_lo=g_lo, g_hi=g_hi, G_CORE=G_CORE,
    )


def _build(meta, W):
    NODES_PAD = meta["NODES_PAD"]
    NT = meta["NT"]
    NCHUNK = meta["NCHUNK"]
    TABLE_ROWS = meta["TABLE_ROWS"]
    NCOL = meta["NCOL"]
    NPB = meta["NPB"]
    G_PAD = meta["G_PAD"]
    GT = meta["GT"]
    N_GLOBAL = meta["N"]
    NF = float(meta["NF_TOTAL"])
    groups = meta["groups"]
    pool_by_gt = meta["pool_by_gt"]

    nc = bacc.Bacc("TRN2", target_bir_lowering=False, debug=False,
                   num_devices=NCORES, num_swdge_queues=4)

    # ---- I/O ----
    nf_in = nc.dram_tensor("nf", [NODES_PAD, DIN], BF16, kind="ExternalInput")
    gidx_in = nc.dram_tensor("gidx", [16, NCOL * 8], I16, kind="ExternalInput")
    dmap_in = nc.dram_tensor("dmap", [128, NCOL], BF16, kind="ExternalInput")
    gmap_in = nc.dram_tensor("gmap", [128, NPB], BF16, kind="ExternalInput")

    # weights: identical on all cores -> NEFF consts (no per-call transport)
    wemb_in = nc.inline_tensor(W["wemb"], name="wemb")
    bemb_in = nc.inline_tensor(W["bemb"], name="bemb")
    wc_in = nc.inline_tensor(W["wc"], name="wc")
    wr_in = nc.inline_tensor(W["wr"], name="wr")
    cb_in = nc.inline_tensor(W["cb"], name="cb")
    rb_in = nc.inline_tensor(W["rb"], name="rb")
    bng_in = nc.inline_tensor(W["bng"], name="bng")
    bnb_in = nc.inline_tensor(W["bnb"], name="bnb")
    w1_in = nc.inline_tensor(W["w1"], name="w1")
    b1_in = nc.inline_tensor(W["b1"], name="b1")
    w2_in = nc.inline_tensor(W["w2"], name="w2")
    b2_in = nc.inline_tensor(W["b2"], name="b2")

    out_dram = nc.dram_tensor("out", [1, G_PAD], F32, kind="ExternalOutput")

    S_tab = nc.dram_tensor("stab", [NCOL * 128, 128], BF16)
    y_bounce = [nc.dram_tensor(f"yb{l}", [NODES_PAD, HP], BF16) for l in range(L)]
    y_full = [nc.dram_tensor(f"yf{l}", [TABLE_ROWS, HP], BF16, addr_space="Shared")
              for l in range(L)]
    st_in = [nc.dram_tensor(f"sti{l}", [2 * 128, 2], F32) for l in range(L)]
    st_out = [nc.dram_tensor(f"sto{l}", [2 * 128, 2], F32, addr_space="Shared")
              for l in range(L)]
    RG = [list(range(NCORES))]

    max_gcols = max(max(nc_ for _, _, nc_ in g["c_slices"]) for g in groups)
    max_tgcols = max(g["ncols"] for g in groups)
    NJ = NODES_PAD // 512  # 512-col chunks

    with tile.TileContext(nc) as tc:
        nc.gpsimd.load_library(_mlp_lib)
        with (
            tc.tile_pool(name="state", bufs=1) as stp,
            tc.tile_pool(name="const", bufs=1) as cst,
            tc.tile_pool(name="work", bufs=3) as wk,
            tc.tile_pool(name="gath", bufs=2) as gp,
            tc.tile_pool(name="spool", bufs=2) as sp,
            tc.tile_pool(name="psy", bufs=2, space="PSUM") as psy,
            tc.tile_pool(name="psagg", bufs=2, space="PSUM") as psagg,
            tc.tile_pool(name="pst", bufs=2, space="PSUM") as pst,
            tc.tile_pool(name="psr", bufs=2, space="PSUM") as psr,
        ):
            ident = cst.tile([128, 128], BF16, tag="ident")
            make_identity(nc, ident[:])

            # iota row 0..127 (f32, same on every partition)
            it16 = cst.tile([128, 128], I16, tag="it16")
            nc.gpsimd.iota(it16[:], pattern=[[1, 128]], base=0, channel_multiplier=0)
            iota_bf = cst.tile([128, 128], F32, tag="iotabf")
            nc.vector.tensor_copy(out=iota_bf[:], in_=it16[:])

            # ---- load weights ----
            wemb = cst.tile([DIN, H], BF16, tag="wemb")
            nc.sync.dma_start(out=wemb[:], in_=wemb_in[:])
            bemb = []
            for m, (f0, fw) in enumerate(FCH):
                t_ = cst.tile([fw, 1], F32, tag=f"bemb{m}", name=f"bemb{m}")
                nc.sync.dma_start(out=t_[:], in_=bemb_in[f0:f0 + fw, :])
                bemb.append(t_)
            w1 = []
            for k, (f0, fw) in enumerate(FCH):
                t_ = cst.tile([fw, 1024], BF16, tag=f"w1_{k}", name=f"w1_{k}")
                nc.sync.dma_start(out=t_[:], in_=w1_in[f0:f0 + fw, :])
                w1.append(t_)
            b1 = cst.tile([128, 8], F32, tag="b1")
            nc.sync.dma_start(out=b1[:], in_=b1_in.ap().rearrange("(a p) one -> p (a one)", p=128))
            w2 = cst.tile([128, 8], BF16, tag="w2")
            nc.sync.dma_start(out=w2[:], in_=w2_in.ap().rearrange("(a p) one -> p (a one)", p=128))
            b2 = cst.tile([1, 1], F32, tag="b2")
            nc.sync.dma_start(out=b2[:], in_=b2_in[:])

            # ---- resident gather indices: [16, NCOL*8] -> [128, NCOL*8] ----
            idxT = stp.tile([128, NCOL * 8], I16, tag="idxT", name="idxT")
            for k in range(8):
                nc.sync.dma_start(out=idxT[16 * k:16 * (k + 1), :], in_=gidx_in[:])

            # ---- dst map + S table build (once) ----
            dmap_ld = wk.tile([128, NCOL], BF16, tag="dmapld", name="dmapld")
            nc.sync.dma_start(out=dmap_ld[:], in_=dmap_in[:])
            dmap_bf = stp.tile([128, NCOL], F32, tag="dmapbf", name="dmapbf")
            nc.vector.tensor_copy(out=dmap_bf[:], in_=dmap_ld[:])
            gmap_ld = wk.tile([128, NPB], BF16, tag="gmapld", name="gmapld")
            nc.sync.dma_start(out=gmap_ld[:], in_=gmap_in[:])
            gmap_bf = stp.tile([128, NPB], F32, tag="gmapbf", name="gmapbf")
            nc.vector.tensor_copy(out=gmap_bf[:], in_=gmap_ld[:])

            SBC = 8
            for col0 in range(0, NCOL, SBC):
                bc = min(SBC, NCOL - col0)
                sb = wk.tile([128, SBC, 128], BF16, tag="sbuild")
                for c in range(bc):
                    nc.vector.tensor_scalar(
                        out=sb[:, c, :], in0=iota_bf[:],
                        scalar1=dmap_bf[:, col0 + c:col0 + c + 1], scalar2=None,
                        op0=ALU.is_equal)
                nc.sync.dma_start(
                    out=S_tab[col0 * 128:(col0 + bc) * 128, :]
                    .rearrange("(b p) m -> p b m", p=128),
                    in_=sb[:, :bc, :])

            def per_layer_consts(l):
                out = {}
                for nm, src_t in (("wc", wc_in), ("wr", wr_in)):
                    chunks = []
                    for k, (f0, fw) in enumerate(FCH):
                        t_ = wk.tile([fw, H], BF16, tag=f"{nm}k{k}", name=f"{nm}k{k}")
                        nc.sync.dma_start(out=t_[:], in_=src_t[l * H + f0: l * H + f0 + fw, :])
                        chunks.append(t_)
                    out[nm] = chunks
                for nm, src_t in (("cb", cb_in), ("rb", rb_in), ("bng", bng_in), ("bnb", bnb_in)):
                    chunks = []
                    for m, (f0, fw) in enumerate(FCH):
                        t_ = wk.tile([fw, 1], F32, tag=f"{nm}m{m}", name=f"{nm}m{m}")
                        nc.sync.dma_start(out=t_[:], in_=src_t[l * H + f0: l * H + f0 + fw, :])
                        chunks.append(t_)
                    out[nm] = chunks
                return out

            # ---- state ----
            xT = [stp.tile([128, NODES_PAD], BF16, tag="xA0", name="xA0"),
                  stp.tile([72, NODES_PAD], BF16, tag="xA1", name="xA1")]
            z = [stp.tile([128, 1], BF16, tag="z0", name="z0"),
                 stp.tile([72, 1], BF16, tag="z1", name="z1")]

            # ---- embedding: xT = (node_feat @ emb_W + emb_b)^T ----
            for j in range(NJ):
                nfT = wk.tile([DIN, 512], BF16, tag="nfT")
                for q in range(4):
                    nt = j * 4 + q
                    nft = wk.tile([128, DIN], BF16, tag="nft")
                    nc.sync.dma_start(out=nft[:], in_=nf_in[nt * 128:(nt + 1) * 128, :])
                    ptt = pst.tile([128, 128], BF16, tag="pt")
                    nc.tensor.transpose(out=ptt[:DIN, :], in_=nft[:], identity=ident[:])
                    nc.vector.tensor_copy(out=nfT[:, q * 128:(q + 1) * 128], in_=ptt[:DIN, :])
                for m, (f0, fw) in enumerate(FCH):
                    ps = psr.tile([128, 512], F32, tag="pr")
                    nc.tensor.matmul(ps[:fw, :], lhsT=wemb[:, f0:f0 + fw], rhs=nfT[:],
                                     start=True, stop=True)
                    nc.scalar.activation(out=xT[m][:, j * 512:(j + 1) * 512], in_=ps[:fw, :],
                                         func=AF.Identity, bias=bemb[m][:], scale=1.0)
            for m in range(2):
                nc.vector.tensor_copy(out=z[m][:], in_=bemb[m][:])

            # ---- layers ----
            NL = int(os.environ.get("GCN_LAYERS", str(L)))
            SKIP_GATHER = os.environ.get("GCN_NO_GATHER", "0") == "1"
            SKIP_CC = os.environ.get("GCN_NO_CC", "0") == "1"
            STAGE = int(os.environ.get("GCN_STAGE", "9"))
            for l in range(NL):
                C = per_layer_consts(l)
                # y = x @ Wc (node-major), write to DRAM
                for nt in range(NT):
                    ps = psy.tile([128, H], F32, tag="py")
                    for k, (f0, fw) in enumerate(FCH):
                        nc.tensor.matmul(ps[:], lhsT=xT[k][:, nt * 128:(nt + 1) * 128],
                                         rhs=C["wc"][k][:], start=(k == 0), stop=(k == 1))
                    yt = wk.tile([128, HP], BF16, tag="yt")
                    nc.scalar.activation(out=yt[:, :H], in_=ps[:], func=AF.Copy, bias=0.0, scale=1.0)
                    nc.vector.memset(yt[:, H:], 0.0)
                    nc.sync.dma_start(out=y_bounce[l][nt * 128:(nt + 1) * 128, :], in_=yt[:])

                if not SKIP_CC:
                    nc.gpsimd.collective_compute(
                        "AllGather", ALU.bypass, replica_groups=RG,
                        ins=[y_bounce[l].ap().opt()], outs=[y_full[l].ap().opt()])

                # residual: x <- relu(x @ Wr + br)^T in place (y already used x)
                for j in range(NJ if STAGE >= 2 else 0):
                    pss = []
                    for m, (f0, fw) in enumerate(FCH):
                        psm = psr.tile([128, 512], F32, tag="pr", name=f"psm{m}")
                        for k, (kf0, kfw) in enumerate(FCH):
                            nc.tensor.matmul(psm[:fw, :], lhsT=C["wr"][k][:, f0:f0 + fw],
                                             rhs=xT[k][:, j * 512:(j + 1) * 512],
                                             start=(k == 0), stop=(k == 1))
                        pss.append(psm)
                    for m, (f0, fw) in enumerate(FCH):
                        nc.scalar.activation(out=xT[m][:, j * 512:(j + 1) * 512],
                                             in_=pss[m][:fw, :],
                                             func=AF.Relu, bias=C["rb"][m][:], scale=1.0)

                # gather + S-matmul + transpose + relu
                qrr = 0
                for gi, g in enumerate(groups if not SKIP_GATHER else []):
                    ncols_tg = g["ncols"]
                    gt_tile = gp.tile([128, max_tgcols, HP], BF16, tag="gt")
                    S_tg = sp.tile([128, max_tgcols, 128], BF16, tag="stg")
                    nc.sync.dma_start(
                        out=S_tg[:, :ncols_tg, :],
                        in_=S_tab[g["col0"] * 128:(g["col0"] + ncols_tg) * 128, :]
                        .rearrange("(b p) m -> p b m", p=128))
                    for (c, c0, ncols) in g["c_slices"]:
                        if ncols == 0:
                            continue
                        pos0 = c0 - g["col0"]
                        rows_c = min(CHUNK, TABLE_ROWS - c * CHUNK)
                        nc.gpsimd.dma_gather(
                            gt_tile[:, pos0:pos0 + ncols, :],
                            y_full[l][c * CHUNK:c * CHUNK + rows_c, :],
                            idxT[:, c0 * 8:(c0 + ncols) * 8],
                            ncols * 128, ncols * 128, HP,
                            single_packet=False, queue_num=qrr % 4)
                        qrr += 1
                    for t, blocks in g["tile_blocks"].items():
                        ps = psagg.tile([128, H], F32, tag="pa")
                        nb = len(blocks)
                        for bi, colg in enumerate(blocks):
                            pos = colg - g["col0"]
                            nc.tensor.matmul(
                                ps[:], lhsT=S_tg[:, pos, :], rhs=gt_tile[:, pos, :H],
                                start=(bi == 0), stop=(bi == nb - 1))
                        a_sb = wk.tile([128, H], BF16, tag="asb")
                        nc.vector.tensor_copy(out=a_sb[:], in_=ps[:])
                        for m, (f0, fw) in enumerate(FCH):
                            ptt = pst.tile([128, 128], BF16, tag="pt")
                            nc.tensor.transpose(out=ptt[:fw, :], in_=a_sb[:, f0:f0 + fw],
                                                identity=ident[:])
                            ht = wk.tile([128, 128], BF16, tag="ht", bufs=2)
                            nc.scalar.activation(
                                out=ht[:fw, :], in_=ptt[:fw, :],
                                func=AF.Relu, bias=C["cb"][m][:], scale=1.0)
                            nc.vector.tensor_add(
                                out=xT[m][:, t * 128:(t + 1) * 128],
                                in0=xT[m][:, t * 128:(t + 1) * 128], in1=ht[:fw, :])

                # ghost column: z_h = relu(bc) + relu(Wr^T z + br)
                zh = []
                for m, (f0, fw) in enumerate(FCH if STAGE >= 3 else []):
                    ps = psy.tile([128, H], F32, tag="py")
                    for k, (kf0, kfw) in enumerate(FCH):
                        nc.tensor.matmul(ps[:fw, :1], lhsT=C["wr"][k][:, f0:f0 + fw],
                                         rhs=z[k][:], start=(k == 0), stop=(k == 1))
                    zr = wk.tile([128, 1], BF16, tag=f"zr{m}")
                    nc.scalar.activation(out=zr[:fw, :], in_=ps[:fw, :1], func=AF.Relu,
                                         bias=C["rb"][m][:], scale=1.0)
                    zc = wk.tile([128, 1], BF16, tag=f"zc{m}")
                    nc.scalar.activation(out=zc[:fw, :], in_=C["cb"][m][:], func=AF.Relu,
                                         bias=0.0, scale=1.0)
                    zt = wk.tile([128, 1], BF16, tag=f"zh{m}")
                    nc.vector.tensor_add(out=zt[:fw, :], in0=zr[:fw, :], in1=zc[:fw, :])
                    zh.append(zt)

                # BN stats
                sacc = [wk.tile([128, NJ], F32, tag=f"sacc{m}", name=f"sacc{m}") for m in range(2)]
                qacc = [wk.tile([128, NJ], F32, tag=f"qacc{m}", name=f"qacc{m}") for m in range(2)]
                for j in range(NJ if STAGE >= 4 else 0):
                    for m, (f0, fw) in enumerate(FCH):
                        hsl = xT[m][:, j * 512:(j + 1) * 512]
                        nc.scalar.activation(out=hsl, in_=hsl, func=AF.Copy, bias=0.0,
                                             scale=1.0, accum_out=sacc[m][:fw, j:j + 1])
                        scr = wk.tile([128, 512], BF16, tag="scr", bufs=2)
                        nc.scalar.activation(out=scr[:fw, :], in_=hsl, func=AF.Square,
                                             bias=0.0, scale=1.0,
                                             accum_out=qacc[m][:fw, j:j + 1])
                stat = [wk.tile([128, 2], F32, tag=f"stat{m}", name=f"stat{m}") for m in range(2)]
                for m, (f0, fw) in enumerate(FCH if STAGE >= 4 else []):
                    nc.scalar.activation(out=sacc[m][:fw, :], in_=sacc[m][:fw, :],
                                         func=AF.Copy, bias=0.0, scale=1.0,
                                         accum_out=stat[m][:fw, 0:1])
                    nc.scalar.activation(out=qacc[m][:fw, :], in_=qacc[m][:fw, :],
                                         func=AF.Copy, bias=0.0, scale=1.0,
                                         accum_out=stat[m][:fw, 1:2])
                    nc.sync.dma_start(out=st_in[l][m * 128:m * 128 + fw, :], in_=stat[m][:fw, :])
                if STAGE >= 4:
                    if not SKIP_CC:
                        nc.gpsimd.collective_compute(
                            "AllReduce", ALU.add, replica_groups=RG,
                            ins=[st_in[l].ap().opt()], outs=[st_out[l].ap().opt()])
                    else:
                        nc.sync.dma_start(out=st_out[l][:], in_=st_in[l][:])

                # finalize scale/shift per feature chunk
                scale = [wk.tile([128, 1], F32, tag=f"scale{m}", name=f"scale{m}") for m in range(2)]
                shift = [wk.tile([128, 1], F32, tag=f"shift{m}", name=f"shift{m}") for m in range(2)]
                for m, (f0, fw) in enumerate(FCH if STAGE >= 5 else []):
                    st = wk.tile([128, 2], F32, tag=f"sin{m}")
                    nc.sync.dma_start(out=st[:fw, :], in_=st_out[l][m * 128:m * 128 + fw, :])
                    zf = wk.tile([128, 1], F32, tag=f"zf{m}")
                    nc.vector.tensor_copy(out=zf[:fw, :], in_=zh[m][:fw, :])
                    t1 = wk.tile([128, 6], F32, tag=f"tmp{m}")
                    # t1[:,0] = sum_real; t1[:,1] = mean
                    nc.vector.tensor_scalar(out=t1[:fw, 0:1], in0=zf[:fw, :], scalar1=NF,
                                            scalar2=None, op0=ALU.mult)
                    nc.vector.tensor_tensor(out=t1[:fw, 0:1], in0=st[:fw, 0:1],
                                            in1=t1[:fw, 0:1], op=ALU.subtract)
                    nc.vector.tensor_scalar(out=t1[:fw, 1:2], in0=t1[:fw, 0:1],
                                            scalar1=1.0 / N_GLOBAL, scalar2=None, op0=ALU.mult)
                    # t1[:,2] = sumsq_real/N ; t1[:,3] = var + eps
                    nc.vector.tensor_tensor(out=t1[:fw, 2:3], in0=zf[:fw, :], in1=zf[:fw, :],
                                            op=ALU.mult)
                    nc.vector.tensor_scalar(out=t1[:fw, 2:3], in0=t1[:fw, 2:3], scalar1=NF,
                                            scalar2=None, op0=ALU.mult)
                    nc.vector.tensor_tensor(out=t1[:fw, 2:3], in0=st[:fw, 1:2],
                                            in1=t1[:fw, 2:3], op=ALU.subtract)
                    nc.vector.tensor_scalar(out=t1[:fw, 2:3], in0=t1[:fw, 2:3],
                                            scalar1=1.0 / N_GLOBAL, scalar2=None, op0=ALU.mult)
                    nc.vector.tensor_tensor(out=t1[:fw, 3:4], in0=t1[:fw, 1:2],
                                            in1=t1[:fw, 1:2], op=ALU.mult)
                    nc.vector.tensor_tensor(out=t1[:fw, 3:4], in0=t1[:fw, 2:3],
                                            in1=t1[:fw, 3:4], op=ALU.subtract)
                    nc.vector.tensor_scalar(out=t1[:fw, 3:4], in0=t1[:fw, 3:4], scalar1=EPS,
                                            scalar2=None, op0=ALU.add)
                    # t1[:,4] = sqrt(var+eps); t1[:,5] = 1/sqrt
                    nc.scalar.activation(out=t1[:fw, 4:5], in_=t1[:fw, 3:4], func=AF.Sqrt,
                                         bias=0.0, scale=1.0)
                    nc.vector.reciprocal(out=t1[:fw, 5:6], in_=t1[:fw, 4:5])
                    nc.vector.tensor_tensor(out=scale[m][:fw, :], in0=C["bng"][m][:],
                                            in1=t1[:fw, 5:6], op=ALU.mult)
                    nc.vector.tensor_tensor(out=shift[m][:fw, :], in0=t1[:fw, 1:2],
                                            in1=scale[m][:fw, :], op=ALU.mult)
                    nc.vector.tensor_tensor(out=shift[m][:fw, :], in0=C["bnb"][m][:],
                                            in1=shift[m][:fw, :], op=ALU.subtract)
                # apply BN in place on xT
                for j in range(NJ if STAGE >= 5 else 0):
                    for m, (f0, fw) in enumerate(FCH):
                        hsl = xT[m][:, j * 512:(j + 1) * 512]
                        nc.vector.tensor_scalar(out=hsl, in0=hsl, scalar1=scale[m][:fw, :],
                                                scalar2=shift[m][:fw, :],
                                                op0=ALU.mult, op1=ALU.add)
                for m, (f0, fw) in enumerate(FCH if STAGE >= 5 else []):
                    nc.vector.tensor_scalar(out=z[m][:fw, :], in0=zh[m][:fw, :],
                                            scalar1=scale[m][:fw, :], scalar2=shift[m][:fw, :],
                                            op0=ALU.mult, op1=ALU.add)

            # ---- pooling ----
            pooledT = [stp.tile([128, G_PAD], BF16, tag="plT0", name="plT0"),
                       stp.tile([72, G_PAD], BF16, tag="plT1", name="plT1")]
            for gt in range(GT):
                blocks = pool_by_gt[gt]
                pp = psy.tile([128, H], F32, tag="py")
                for bi, (b, t) in enumerate(blocks):
                    xnm = wk.tile([128, H], BF16, tag="xnm")
                    for m, (f0, fw) in enumerate(FCH):
                        ptt = pst.tile([128, 128], BF16, tag="pt")
                        nc.tensor.transpose(out=ptt[:128, :fw],
                                            in_=xT[m][:, t * 128:(t + 1) * 128],
                                            identity=ident[:fw, :fw])
                        nc.vector.tensor_copy(out=xnm[:, f0:f0 + fw], in_=ptt[:128, :fw])
                    Pb = sp.tile([128, 128], BF16, tag="pb")
                    nc.vector.tensor_scalar(out=Pb[:], in0=iota_bf[:],
                                            scalar1=gmap_bf[:, b:b + 1], scalar2=None,
                                            op0=ALU.is_equal)
                    nc.tensor.matmul(pp[:], lhsT=Pb[:], rhs=xnm[:],
                                     start=(bi == 0), stop=(bi == len(blocks) - 1))
                pl_sb = wk.tile([128, H], BF16, tag="plsb")
                nc.vector.tensor_copy(out=pl_sb[:], in_=pp[:])
                for m, (f0, fw) in enumerate(FCH):
                    ptt = pst.tile([128, 128], BF16, tag="pt")
                    nc.tensor.transpose(out=ptt[:fw, :], in_=pl_sb[:, f0:f0 + fw],
                                        identity=ident[:])
                    nc.vector.tensor_copy(out=pooledT[m][:, gt * 128:(gt + 1) * 128],
                                          in_=ptt[:fw, :])

            # ---- MLP ----
            NGC = _ceil(G_PAD, 128)
            for gc in range(NGC):
                gw = 128
                ps2 = psagg.tile([128, H], F32, tag="pa")
                for m8 in range(8):
                    ps = psr.tile([128, 512], F32, tag="pr")
                    for k, (f0, fw) in enumerate(FCH):
                        nc.tensor.matmul(ps[:, :gw], lhsT=w1[k][:, m8 * 128:(m8 + 1) * 128],
                                         rhs=pooledT[k][:, gc * 128:gc * 128 + gw],
                                         start=(k == 0), stop=(k == 1))
                    h1t = wk.tile([128, 512], BF16, tag="h1t", bufs=2)
                    nc.scalar.activation(out=h1t[:, :gw],
                                         in_=ps[:, :gw], func=AF.Relu,
                                         bias=b1[:, m8:m8 + 1], scale=1.0)
                    nc.tensor.matmul(ps2[:1, :gw], lhsT=w2[:, m8:m8 + 1],
                                     rhs=h1t[:, :gw],
                                     start=(m8 == 0), stop=(m8 == 7))
                o_sb = wk.tile([1, 512], F32, tag="osb", bufs=1)
                nc.vector.tensor_scalar(out=o_sb[:1, :gw], in0=ps2[:1, :gw],
                                        scalar1=b2[:1, :], scalar2=None, op0=ALU.add)
                nc.sync.dma_start(out=out_dram[0:1, gc * 128:gc * 128 + gw],
                                  in_=o_sb[:1, :gw])

    nc.compile()
    return nc


def kernel(node_feat, src, dst, graph_id, emb_W, emb_b, conv_W, conv_b,
           res_W, res_b, bn_g, bn_b, out_W1, out_b1, out_W2, out_b2):
    node_feat = np.asarray(node_feat, dtype=np.float32)
    G = 4096
    meta = _preprocess(node_feat, src, dst, graph_id, G=G)

    bf = ml_dtypes.bfloat16
    W = dict(
        wemb=np.ascontiguousarray(np.asarray(emb_W, np.float32).astype(bf)),
        wc=np.ascontiguousarray(np.asarray(conv_W, np.float32).reshape(L * H, H).astype(bf)),
        wr=np.ascontiguousarray(np.asarray(res_W, np.float32).reshape(L * H, H).astype(bf)),
        w1=np.ascontiguousarray(np.asarray(out_W1, np.float32).astype(bf)),
        w2=np.ascontiguousarray(np.asarray(out_W2, np.float32).reshape(1024, 1).astype(bf)),
        bemb=np.asarray(emb_b, np.float32).reshape(H, 1),
        cb=np.asarray(conv_b, np.float32).reshape(L * H, 1),
        rb=np.asarray(res_b, np.float32).reshape(L * H, 1),
        bng=np.asarray(bn_g, np.float32).reshape(L * H, 1),
        bnb=np.asarray(bn_b, np.float32).reshape(L * H, 1),
        b1=np.asarray(out_b1, np.float32).reshape(1024, 1),
        b2=np.asarray(out_b2, np.float32).reshape(1, 1),
    )

    nc = _build(meta, W)

    global _last_nc, _last_in_maps
    in_maps = []
    for c in range(NCORES):
        in_maps.append({
            "nf": meta["nf_all"][c], "gidx": meta["idx_all"][c],
            "dmap": meta["dmap_all"][c], "gmap": meta["gmap_all"][c],
        })
    _last_nc, _last_in_maps = nc, in_maps
    res = run_bass_kernel_spmd(nc, in_maps, core_ids=list(range(NCORES)))

    out = np.zeros((G, 1), np.float32)
    for c in range(NCORES):
        gc_ = int(meta["G_CORE"][c])
        vals = res.results[c]["out"][0, :gc_]
        out[meta["g_lo"][c]:meta["g_hi"][c], 0] = vals
    return out
